# revision 1
# baseline (speedup 1.0000x reference)
"""Contrastive Predictive Coding loss kernel for 8 Trainium2 NeuronCores.

Strategy (SPMD, batch row-sharded):
  - B=8192 rows split across 8 cores (1024 rows each). All activations are
    kept TRANSPOSED on chip ([D, batch] with D on partitions) so every matmul
    uses the torch-layout [in,out] weights directly as lhsT with no on-device
    transposes; the raw inputs are transposed (and bf16-cast) on host.
  - Softmax shift-invariance: logits_ij = 10*pred_i.(h1_j @ W2 + b2); the b2
    term is a per-row constant that cancels exactly in LSE_i - logit_ii, and
    pred.(h1 @ W2) = (pred @ W2^T).h1, so the encoder's second layer for
    next_state is never materialized. Each core computes q = pred @ W2^T
    (W2^T supplied by host) and contracts logits against the relu layer-1
    output h1, which every core computes for the FULL batch (redundant
    layer-1 encode is cheaper than an AllGather here).
  - Logits row-block (1024 x 8192) accumulates in PSUM from bf16 matmuls;
    softmax statistics are fused into the ScalarE pass: exp(10*x) with
    accum_out row-sums, no max-subtraction (logits are bounded ~|8|, exp is
    fp32-safe). The final log() runs on host (8192 values).
  - diag(logits) = rowsum(q * h1_local) via fp32 elementwise multiply + a
    matmul against a constant 10.0 vector (partition reduction on PE).
  - Per-row [rowsum, diag] returned as [128, 16] per core; host finishes
    loss = mean(log(rowsum) - diag).
"""

import os
import sys

import numpy as np

for _p in ("/opt/trn_rl_repo", "/root/.axon_site/_ro/trn_rl_repo"):
    if os.path.isdir(_p) and _p not in sys.path:
        sys.path.append(_p)

D = 256
B = 8192
NCORES = 8
SH = B // NCORES          # 1024 rows per core
KP = D // 128             # 2 partition chunks of the contraction dim
NB = 512                  # matmul moving-operand block (one fp32 PSUM bank)
RT = SH // 128            # 8 row tiles of 128 pred rows
G = 2048                  # columns per fused exp/accum group (4 PSUM banks)
NG = B // G               # 4 groups per row tile
SCALE = 10.0              # 1 / temperature

_cache = {}


def _build():
    if "nc" in _cache:
        return _cache["nc"]

    import concourse.bacc as bacc
    import concourse.mybir as mybir
    import concourse.tile as tile

    dt = mybir.dt
    f32 = dt.float32
    bf16 = dt.bfloat16
    AF = mybir.ActivationFunctionType
    ALU = mybir.AluOpType
    AX = mybir.AxisListType

    nc = bacc.Bacc("TRN2", target_bir_lowering=False, num_devices=NCORES)

    xsT = nc.declare_dram_parameter("xsT", [D, SH], bf16, isOutput=False)
    xnTl = nc.declare_dram_parameter("xnTl", [D, SH], bf16, isOutput=False)
    xnTf = nc.declare_dram_parameter("xnTf", [D, B], bf16, isOutput=False)
    w_e1 = nc.declare_dram_parameter("w_e1", [D, D], bf16, isOutput=False)
    w_e2 = nc.declare_dram_parameter("w_e2", [D, D], bf16, isOutput=False)
    w_e2T = nc.declare_dram_parameter("w_e2T", [D, D], bf16, isOutput=False)
    w_g = nc.declare_dram_parameter("w_g", [D, 3 * D], bf16, isOutput=False)
    w_p1 = nc.declare_dram_parameter("w_p1", [D, D], bf16, isOutput=False)
    w_p2 = nc.declare_dram_parameter("w_p2", [D, D], bf16, isOutput=False)
    # bias columns: 0=enc_b1 1=enc_b2 2=gru_r 3=gru_z 4=gru_n(bih) 5=gru_n(bhh)
    #               6=pred_b1 7=pred_b2
    bpk = nc.declare_dram_parameter("bpk", [D, 8], f32, isOutput=False)
    # out columns 0..7 = exp-row-sums per row tile; out2 = raw diag dots
    out_d = nc.declare_dram_parameter("out", [128, 2 * RT], f32, isOutput=True)
    out2_d = nc.declare_dram_parameter("out2", [1, SH], f32, isOutput=True)

    with tile.TileContext(nc, num_cores=NCORES) as tc:
        with (
            tc.tile_pool(name="persist", bufs=1) as pp,
            tc.tile_pool(name="scratch", bufs=8) as sp,
            tc.tile_pool(name="small", bufs=4) as smp,
        ):
            # ---- inputs in critical-path order ---------------------------
            # first matmul needs we1 + xs; first epilogue needs bp
            wt = {}

            def load_w(name, hdl, width):
                wt[name] = []
                for k in range(KP):
                    t = pp.tile([128, width], bf16, name=f"{name}_{k}")
                    nc.sync.dma_start(out=t[:, :], in_=hdl[k * 128 : (k + 1) * 128, :])
                    wt[name].append(t)

            load_w("we1", w_e1, D)  # sync queue: critical path
            bp = []
            for k in range(KP):  # scalar queue: bias needed by first epilogue
                t = pp.tile([128, 8], f32, name=f"bp_{k}")
                nc.scalar.dma_start(out=t[:, :], in_=bpk[k * 128 : (k + 1) * 128, :])
                bp.append(t)
            xs, xnl = [], []
            for k in range(KP):
                t = pp.tile([128, SH], bf16, name=f"xs_{k}")
                nc.sync.dma_start(out=t[:, :], in_=xsT[k * 128 : (k + 1) * 128, :])
                xs.append(t)

            # full next_state on the gpsimd (SWDGE) queue, chunked so
            # layer-1 can start on chunk 0 while later chunks stream
            xnf = [pp.tile([128, B], bf16, name=f"xnf_{k}") for k in range(KP)]
            XCH = 2048
            for c0 in range(0, B, XCH):
                for k in range(KP):
                    nc.gpsimd.dma_start(
                        out=xnf[k][:, c0 : c0 + XCH],
                        in_=xnTf[k * 128 : (k + 1) * 128, c0 : c0 + XCH],
                    )

            load_w("we2", w_e2, D)
            load_w("wg", w_g, 3 * D)
            load_w("wp1", w_p1, D)
            load_w("wp2", w_p2, D)
            load_w("we2T", w_e2T, D)
            for k in range(KP):
                t = pp.tile([128, SH], bf16, name=f"xnl_{k}")
                nc.sync.dma_start(out=t[:, :], in_=xnTl[k * 128 : (k + 1) * 128, :])
                xnl.append(t)
            ones = pp.tile([128, 1], bf16, name="ones")
            nc.vector.memset(ones[:, :], 1.0)

            # persistent activations
            h1f = [pp.tile([128, B], bf16, name=f"h1f_{k}") for k in range(KP)]
            qT = [pp.tile([128, SH], bf16, name=f"qT_{k}") for k in range(KP)]
            dummy = pp.tile([128, G], f32, name="dummy")
            out_sb = pp.tile([128, 2 * RT], f32, name="out_sb")

            with tc.tile_pool(name="psx", bufs=2, space="PSUM") as psx:
                # Single shared PSUM pool: every tile uses the same tag so
                # the whole kernel rotates through two 4-bank slots with no
                # pool-transition barrier before the logits phase.
                def ptile(name):
                    return psx.tile([128, G], f32, name=name, tag="px")

                def layer(dst, src, w, bias_col, act, tag, split="split"):
                    """dst[m] = act(w.T @ src + b) over SH cols; the two
                    128-row output chunks epilogue on ScalarE (m=0) and
                    VectorE (m=1) so they drain in parallel."""
                    for m in range(KP):
                        ps = ptile(f"ps_{tag}_{m}")
                        for nb in range(SH // NB):
                            sl = slice(nb * NB, (nb + 1) * NB)
                            for k in range(KP):
                                nc.tensor.matmul(
                                    ps[:, sl],
                                    lhsT=w[k][:, m * 128 : (m + 1) * 128],
                                    rhs=src[k][:, sl],
                                    start=(k == 0),
                                    stop=(k == KP - 1),
                                )
                        on_act = (m == 0) if split == "split" else (split == "act")
                        bias = None if bias_col is None else bp[m][
                            :, bias_col : bias_col + 1
                        ]
                        if act == "relu" and on_act:
                            nc.scalar.activation(
                                out=dst[m][:, :], in_=ps[:, :SH],
                                func=AF.Relu, bias=bias,
                            )
                        elif act == "relu":
                            nc.vector.tensor_scalar(
                                out=dst[m][:, :], in0=ps[:, :SH],
                                scalar1=bias, scalar2=0.0,
                                op0=ALU.add, op1=ALU.max,
                            )
                        elif act == "bias" and on_act:
                            nc.scalar.activation(
                                out=dst[m][:, :], in_=ps[:, :SH],
                                func=AF.Identity, bias=bias,
                            )
                        elif act == "bias":
                            nc.vector.tensor_scalar(
                                out=dst[m][:, :], in0=ps[:, :SH],
                                scalar1=bias, scalar2=None, op0=ALU.add,
                            )
                        elif on_act:  # copy
                            nc.scalar.copy(dst[m][:, :], ps[:, :SH])
                        else:
                            nc.vector.tensor_copy(dst[m][:, :], ps[:, :SH])

                # layer-1 over the FULL batch, emitted one [128, G] group at
                # a time between chain stages so the PE's static program has
                # ready work wherever the latency-bound chain stalls.
                _l1_state = {"i": 0}

                def emit_l1(n=1):
                    for _ in range(n):
                        i = _l1_state["i"]
                        _l1_state["i"] += 1
                        if i >= 2 * (B // G):
                            return
                        m, g0 = i % KP, (i // KP) * G
                        ps = ptile(f"ps_l1_{i}")
                        for s in range(G // NB):
                            sl = slice(s * NB, (s + 1) * NB)
                            gsl = slice(g0 + s * NB, g0 + (s + 1) * NB)
                            for k in range(KP):
                                nc.tensor.matmul(
                                    ps[:, sl],
                                    lhsT=wt["we1"][k][:, m * 128 : (m + 1) * 128],
                                    rhs=xnf[k][:, gsl],
                                    start=(k == 0),
                                    stop=(k == KP - 1),
                                )
                        dsl = slice(g0, g0 + G)
                        nc.vector.tensor_scalar(
                            out=h1f[m][:, dsl], in0=ps[:, :],
                            scalar1=bp[m][:, 0:1], scalar2=0.0,
                            op0=ALU.add, op1=ALU.max,
                        )

                # ---- local chain: z_t -> GRU -> pred -> q ----------------
                h1s = [sp.tile([128, SH], bf16, name=f"h1s_{k}", tag="scr") for k in range(KP)]
                zt = [sp.tile([128, SH], bf16, name=f"zt_{k}", tag="scr") for k in range(KP)]
                layer(h1s, xs, wt["we1"], 0, "relu", "h1s")
                layer(zt, h1s, wt["we2"], 1, "bias", "zt")
                # local next-state relu layer (for the diagonal), off the
                # critical path: fills the PE stall behind zt's epilogue
                h1nl = [sp.tile([128, SH], bf16, name=f"h1nl_{k}", tag="scr") for k in range(KP)]
                layer(h1nl, xnl, wt["we1"], 0, "relu", "h1nl", split="dve")

                def gate_psum(gate, m, tag):
                    ps = ptile(f"ps_{tag}_{m}")
                    col0 = gate * D + m * 128
                    for nb in range(SH // NB):
                        sl = slice(nb * NB, (nb + 1) * NB)
                        for k in range(KP):
                            nc.tensor.matmul(
                                ps[:, sl],
                                lhsT=wt["wg"][k][:, col0 : col0 + 128],
                                rhs=zt[k][:, sl],
                                start=(k == 0),
                                stop=(k == KP - 1),
                            )
                    return ps

                rg = [sp.tile([128, SH], f32, name=f"rg_{m}", tag="scr") for m in range(KP)]
                zg = [sp.tile([128, SH], f32, name=f"zg_{m}", tag="scr") for m in range(KP)]
                ng = [sp.tile([128, SH], f32, name=f"ng_{m}", tag="scr") for m in range(KP)]
                ctx = [sp.tile([128, SH], bf16, name=f"ctx_{m}", tag="scr") for m in range(KP)]
                for m in range(KP):
                    ps_r = gate_psum(0, m, "gr")
                    nc.scalar.activation(
                        out=rg[m][:, :], in_=ps_r[:, :SH], func=AF.Sigmoid,
                        bias=bp[m][:, 2:3],
                    )
                for m in range(KP):
                    ps_z = gate_psum(1, m, "gz")
                    nc.scalar.activation(
                        out=zg[m][:, :], in_=ps_z[:, :SH], func=AF.Sigmoid,
                        bias=bp[m][:, 3:4],
                    )
                emit_l1()  # first logits block needs h1f[:, 0:G]
                for m in range(KP):
                    ps_n = gate_psum(2, m, "gn")
                    tmp = sp.tile([128, SH], f32, name=f"tmp_{m}", tag="scr")
                    # tmp = r * bhh_n + gi_n   (one fused DVE op)
                    nc.vector.scalar_tensor_tensor(
                        out=tmp[:, :], in0=rg[m][:, :], scalar=bp[m][:, 5:6],
                        in1=ps_n[:, :SH], op0=ALU.mult, op1=ALU.add,
                    )
                    nc.scalar.activation(
                        out=ng[m][:, :], in_=tmp[:, :], func=AF.Tanh,
                        bias=bp[m][:, 4:5],
                    )
                    omz = sp.tile([128, SH], f32, name=f"omz_{m}", tag="scr")
                    nc.vector.tensor_scalar(
                        out=omz[:, :], in0=zg[m][:, :],
                        scalar1=-1.0, scalar2=1.0, op0=ALU.mult, op1=ALU.add,
                    )
                    nc.vector.tensor_tensor(
                        out=ctx[m][:, :], in0=omz[:, :], in1=ng[m][:, :], op=ALU.mult
                    )
                emit_l1()

                h1p = [sp.tile([128, SH], bf16, name=f"h1p_{k}", tag="scr") for k in range(KP)]
                pr = [sp.tile([128, SH], bf16, name=f"pr_{k}", tag="scr") for k in range(KP)]
                layer(h1p, ctx, wt["wp1"], 6, "relu", "h1p")
                layer(pr, h1p, wt["wp2"], 7, "bias", "pr")
                # q = pred @ W2^T  (no bias: enc_b2 cancels in LSE - diag)
                layer(qT, pr, wt["we2T"], None, "copy", "q")
                # ---- diag: rowsum(q * h1_local) via ones-vector matmul ---
                prod = [sp.tile([128, SH], bf16, name=f"prod_{k}", tag="scr") for k in range(KP)]
                for k in range(KP):
                    nc.vector.tensor_tensor(
                        out=prod[k][:, :], in0=qT[k][:, :], in1=h1nl[k][:, :],
                        op=ALU.mult,
                    )

                # ---- logits, column-block outer: block g's exp stream on
                # ScalarE hides block g+1's layer-1 matmuls on the PE -------
                sums = pp.tile([128, RT * NG], f32, name="sums")
                for g in range(NG):
                    for t in range(RT):
                        if t == 2:
                            emit_l1(2)  # h1f block g+1 while block g streams
                        pl = ptile("pl")
                        for s in range(G // NB):
                            c0 = g * G + s * NB
                            sl = slice(s * NB, (s + 1) * NB)
                            for k in range(KP):
                                nc.tensor.matmul(
                                    pl[:, sl],
                                    lhsT=qT[k][:, t * 128 : (t + 1) * 128],
                                    rhs=h1f[k][:, c0 : c0 + NB],
                                    start=(k == 0),
                                    stop=(k == KP - 1),
                                )
                        nc.scalar.activation(
                            out=dummy[:, :], in_=pl[:, :], func=AF.Exp,
                            scale=SCALE, accum_out=sums[:, t * NG + g : t * NG + g + 1],
                        )
                    if g == 0 and t == RT - 1:
                        dps = ptile("dps")
                        for s in range(SH // NB):
                            sl = slice(s * NB, (s + 1) * NB)
                            for k in range(KP):
                                nc.tensor.matmul(
                                    dps[:1, sl],
                                    lhsT=ones[:, :],
                                    rhs=prod[k][:, sl],
                                    start=(k == 0),
                                    stop=(k == KP - 1),
                                )
                        d2 = pp.tile([1, SH], f32, name="d2")
                        nc.vector.tensor_copy(d2[:, :], dps[:1, :SH])
                        nc.sync.dma_start(out=out2_d[:, :], in_=d2[:, :])
                for t in range(RT):
                    nc.vector.reduce_sum(
                        out=out_sb[:, t : t + 1],
                        in_=sums[:, t * NG : (t + 1) * NG], axis=AX.X,
                    )

            nc.sync.dma_start(out=out_d[:, :], in_=out_sb[:, :])

    if not nc.is_finalized():
        nc.finalize()
    _cache["nc"] = nc
    return nc


def _prep_in_maps(inputs):
    import ml_dtypes

    bf = ml_dtypes.bfloat16
    f = lambda x: np.ascontiguousarray(np.asarray(x), dtype=np.float32)
    state = f(inputs["state"])
    next_state = f(inputs["next_state"])
    bias_pack = np.stack(
        [
            f(inputs["enc_b1"]),
            f(inputs["enc_b2"]),
            f(inputs["gru_bih"])[:D] + f(inputs["gru_bhh"])[:D],
            f(inputs["gru_bih"])[D : 2 * D] + f(inputs["gru_bhh"])[D : 2 * D],
            f(inputs["gru_bih"])[2 * D :],
            f(inputs["gru_bhh"])[2 * D :],
            f(inputs["pred_b1"]),
            f(inputs["pred_b2"]),
        ],
        axis=1,
    )
    w_e2 = f(inputs["enc_w2"])
    nT = np.ascontiguousarray(next_state.T).astype(bf)  # [D, B]
    sT = np.ascontiguousarray(state.T).astype(bf)
    shared = {
        "w_e1": f(inputs["enc_w1"]).astype(bf),
        "w_e2": w_e2.astype(bf),
        "w_e2T": np.ascontiguousarray(w_e2.T).astype(bf),
        "w_g": f(inputs["gru_wih"]).astype(bf),
        "w_p1": f(inputs["pred_w1"]).astype(bf),
        "w_p2": f(inputs["pred_w2"]).astype(bf),
        "bpk": np.ascontiguousarray(bias_pack, dtype=np.float32),
        "xnTf": nT,
    }
    in_maps = []
    for c in range(NCORES):
        sl = slice(c * SH, (c + 1) * SH)
        in_maps.append(
            {
                "xsT": np.ascontiguousarray(sT[:, sl]),
                "xnTl": np.ascontiguousarray(nT[:, sl]),
                **shared,
            }
        )
    return in_maps


last_results = None


def _finish(results):
    total = 0.0
    for r in results:
        rowsum = r["out"].astype(np.float64)[:, :RT]  # [128, RT], col t = tile
        diag = r["out2"].astype(np.float64).reshape(-1)  # [SH] raw dots
        lse_flat = np.log(rowsum).T.reshape(-1)  # row j = t*128+p
        total += float((lse_flat - SCALE * diag).sum())
    return np.float32(total / B)


def kernel(**inputs) -> np.ndarray:
    from concourse.bass_utils import run_bass_kernel_spmd

    global last_results
    nc = _build()
    in_maps = _prep_in_maps(inputs)
    res = run_bass_kernel_spmd(nc, in_maps, core_ids=list(range(NCORES)))
    last_results = res
    return _finish(res.results)


# ---------------------------------------------------------------------------
# Pure-numpy golden model of the exact device algorithm (for test.py).
def golden(**inputs) -> np.ndarray:
    in_maps = _prep_in_maps(inputs)
    f32 = np.float32
    m0 = in_maps[0]
    bfd = m0["w_e1"].dtype

    def as32(x):
        return x.astype(f32)

    # full layer-1 (same on every core)
    h1f = np.maximum(as32(m0["xnTf"]).T @ as32(m0["w_e1"]) + m0["bpk"][:, 0], 0.0)
    h1f = h1f.astype(bfd).astype(f32)  # [B, D]
    results = []
    for c in range(NCORES):
        m = in_maps[c]
        xs = as32(m["xsT"]).T
        h1 = np.maximum(xs @ as32(m["w_e1"]) + m["bpk"][:, 0], 0.0)
        h1 = h1.astype(bfd).astype(f32)
        zt = (h1 @ as32(m["w_e2"]) + m["bpk"][:, 1]).astype(bfd).astype(f32)
        gi = zt @ as32(m["w_g"])
        r = 1.0 / (1.0 + np.exp(-(gi[:, :D] + m["bpk"][:, 2])))
        z = 1.0 / (1.0 + np.exp(-(gi[:, D : 2 * D] + m["bpk"][:, 3])))
        n = np.tanh(gi[:, 2 * D :] + m["bpk"][:, 4] + r * m["bpk"][:, 5])
        ctx = ((1.0 - z) * n).astype(bfd).astype(f32)
        h1p = np.maximum(ctx @ as32(m["w_p1"]) + m["bpk"][:, 6], 0.0)
        h1p = h1p.astype(bfd).astype(f32)
        pred = (h1p @ as32(m["w_p2"]) + m["bpk"][:, 7]).astype(bfd).astype(f32)
        q = (pred @ as32(m["w_e2T"])).astype(bfd).astype(f32)  # [SH, D]
        logits = SCALE * (q @ h1f.T)  # [SH, B]
        rowsum = np.exp(logits).sum(axis=1)
        h1l = np.maximum(as32(m["xnTl"]).T @ as32(m["w_e1"]) + m["bpk"][:, 0], 0.0)
        h1l = h1l.astype(bfd).astype(f32)
        prod = (q * h1l).astype(bfd).astype(f32)
        diag = prod.sum(axis=1)  # raw dot (scaled on host)
        out = np.concatenate(
            [rowsum.reshape(RT, 128).T, np.zeros((128, RT), f32)], axis=1
        )
        results.append(
            {"out": out.astype(np.float32), "out2": diag.reshape(1, SH).astype(np.float32)}
        )
    return _finish(results)

